# revision 58
# baseline (speedup 1.0000x reference)
"""Trainium2 Bass kernel for CompanySpecificHeads (MoE-style routed MLP heads).

Semantics (matching the reference):
    out[b] = gelu(z[b] @ W1[cid[b]] + b1[cid[b]]) @ W2[cid[b]] + b2[cid[b]]

Strategy: expert-parallel across 8 NeuronCores, 8 companies per core, with a
tokens-stationary PE mapping and fp8 weights:

  - W1 is streamed from HBM in fp8 e3m4 (scaled by 64, clipped to +-15.5),
    halving the dominant DMA traffic vs fp16. Measured end-to-end rel err
    ~1.3e-2 (gate 2e-2); z stays fp16 (mixed-dtype matmul is allowed).
  - Layer 1 runs tokens-stationary: the routed z tile [128d x <=64 tokens]
    is the stationary operand, W1[c] k-chunks stream as the N=512 moving
    operand into psum[tokens, h]. ~10 matmuls/company instead of 42, and no
    per-matmul 128-col weight LDWEIGHTS churn.
  - Token chunks of different companies sit at disjoint psum partition
    ranges (column tiling: 2 x 64-wide at pos 0/64, or 4 x 32-wide at pos
    0/32/64/96) so their W1 streams run concurrently on separate XBUSes.
  - b1 enters via a K=1 ones-matmul into psum before the k-chunks.
  - gelu: ACT reads psum with scale=1/64 (undoes the fp8 scaling).
  - Layer 2 (O=1): DVE tensor_tensor_reduce (g * W2rep, reduce-add along h)
    into a staging tile. DVE requires partition base 0/64, so each ttr
    covers a 64-row side with one company's W2 block; rows belonging to
    other companies are garbage and ignored by the host. W2 is replicated
    across partitions by GPSIMD partition_broadcast (idle engine).
  - Host scatters staged outputs back to [B, 1] and adds b2 in fp32.
"""

import numpy as np

B, C, D, H = 4096, 64, 512, 1024
NCORES = 8
CPC = C // NCORES  # companies per core
KC = D // 128      # contraction chunks of 128
S = 64.0           # fp8 weight scale

_COMPILED = {}


def _plan(TW):
    """Slot list; each slot is a list of entries (c, lo, w, pos).

    First-64 token chunks pair two companies at pos 0/64; overflow chunks
    (w<=32) pack four companies at pos 0/32/64/96, (32<w<=64) pack two.
    """
    slots = []
    wA = min(TW, 64)
    for c0 in range(0, CPC, 2):
        slots.append([(c0 + i, 0, wA, 64 * i) for i in range(2) if c0 + i < CPC])
    if TW > 64:
        wB = TW - 64
        if wB <= 32:
            for c0 in range(0, CPC, 4):
                slots.append([(c0 + i, 64, wB, 32 * i)
                              for i in range(4) if c0 + i < CPC])
        else:
            for c0 in range(0, CPC, 2):
                slots.append([(c0 + i, 64, wB, 64 * i)
                              for i in range(2) if c0 + i < CPC])
    return slots


def _entries(TW):
    """Flat (c, lo, w, pos, col) list; col = stage column = slot index."""
    out = []
    for si, slot in enumerate(_plan(TW)):
        for (c, lo, w, pos) in slot:
            out.append((c, lo, w, pos, si))
    return out


def _build(TW):
    import concourse.bass as bass
    import concourse.bacc as bacc
    import concourse.mybir as mybir
    from concourse.tile import TileContext
    from contextlib import ExitStack

    f32 = mybir.dt.float32
    f16 = mybir.dt.float16
    f8 = mybir.dt.float8e3

    slots = _plan(TW)
    NCOL = len(slots)
    NPAIR = (CPC + 1) // 2

    gelu = mybir.ActivationFunctionType.Gelu
    ident = mybir.ActivationFunctionType.Identity
    mult = mybir.AluOpType.mult
    add = mybir.AluOpType.add

    nc = bacc.Bacc(None, target_bir_lowering=False)

    # z tokens routed+padded per company: zt[p, k, c*TW + t] = z[tok, 128k+p]
    zt_d = nc.dram_tensor("zt", [128, KC, CPC * TW], f16, kind="ExternalInput")
    # W1 pair-major: w1[pr, p, cc*KC*H + k*H + h] = W1[2pr+cc, 128k+p, h]*S (e3m4)
    w1_d = nc.dram_tensor("w1", [NPAIR, 128, 2 * KC * H], f8, kind="ExternalInput")
    # consts on partition 0: [b1*S (c-major, CPC*H) | w2 (c-major, CPC*H)]
    cst_d = nc.dram_tensor("cst", [1, 2 * CPC * H], f16, kind="ExternalInput")
    out_d = nc.dram_tensor("out", [128, 2 * NCOL * 16], f16,
                           kind="ExternalOutput")

    with TileContext(nc) as tc, ExitStack() as ctx:
        const = ctx.enter_context(tc.tile_pool(name="const", bufs=1))

        # --- input DMAs ---
        # Everything on ONE HWDGE ring (sync), in need-order: two rings
        # do NOT share SDMA bandwidth fairly (the W1 ring starved z/cst
        # until ~20us), while a single ring is FIFO and hits full rate.
        cst = const.tile([1, 2 * CPC * H], f16)
        nc.sync.dma_start(out=cst[:], in_=cst_d[:])
        b1t = cst[:, 0:CPC * H]
        w2row = cst[:, CPC * H:]

        # z k0-1 now; k2-3 after the first W1 pair (slot 0 starts sooner)
        zt = const.tile([128, KC, CPC * TW], f16)
        nc.sync.dma_start(out=zt[:, :2], in_=zt_d[:, :2])

        # W1: one DMA per company pair (~1MB each, e3m4).
        w1ts = []
        for pr in range(NPAIR):
            w1t = const.tile([128, 2, KC, H], f8, name=f"w1_{pr}")
            nc.sync.dma_start(out=w1t[:], in_=w1_d[pr])
            w1ts.append(w1t)
            if pr == 0:
                nc.sync.dma_start(out=zt[:, 2:], in_=zt_d[:, 2:])

        # --- small constants on-chip ---
        ones = const.tile([1, 128], f16)
        nc.gpsimd.memset(ones[:], 1.0)
        wsc = const.tile([128, 128], f16)
        nc.gpsimd.memset(wsc[:], 0.0)
        wsc2 = const.tile([128, 512], f16)
        nc.gpsimd.memset(wsc2[:], 0.0)

        # 16 fp16 partial sums per (slot, h-half); host finishes the sum
        stage2 = const.tile([128, 2 * NCOL * 16], f16)

        # pools — PSUM: psA(3) + psB(3) + pt(2) tiles of one bank each = 8 banks
        pp = ctx.enter_context(tc.tile_pool(name="pp", bufs=3, space="PSUM"))
        gp = ctx.enter_context(tc.tile_pool(name="gp", bufs=8))
        scr = ctx.enter_context(tc.tile_pool(name="scr", bufs=6))

        # --- PE warmup: keep the PE busy until the first W1 pair lands so
        # the HAM clock-gate reaches (and holds) the warm 2.4GHz state.
        # Dependency-free N=128 matmuls on memset scratch (~214ns each cold
        # including the per-matmul LDWEIGHTS).
        # N=512 keeps the PE ~80% busy despite the per-matmul LDWEIGHTS, so
        # the HAM activity window actually trips before the main stream.
        wp = pp.tile([128, 512], f32, tag="pt", bufs=2)
        for _ in range(10):
            nc.tensor.matmul(wp[:], wsc[:], wsc2[:], start=True, stop=True)

        # --- per-slot W2 tiles: w2slot[s][pos:pos+w, h] = W2[c, h] ---
        # Replicate each company's W2 row across its slot's psum partition
        # range with a K=1 ones-matmul at that tile position (the only
        # partition-range replication that works at runtime), then one
        # full-width DVE copy psum -> SBUF. Construction for slot s+1 is
        # emitted inside the main stream (after slot s's matmuls) so the
        # psum->SBUF copy dependencies never stall the PE queue head.
        w2slot = [const.tile([128, H], f16, name=f"w2slot_{si}")
                  for si in range(len(slots))]

        def build_w2slot(si):
            # early slots' psum->SBUF copies go to DVE (idle before the
            # stream); later slots' to ACT (DVE is the mid-stream critical
            # path, ACT has slack)
            slot = slots[si]
            for bh in range(2):
                pt = pp.tile([128, 512], f32, name=f"w2p{si}_{bh}", tag="pt",
                             bufs=2)
                for (c, lo, w, pos) in slot:
                    nc.tensor.matmul(
                        pt[pos:pos + w, :],
                        ones[:, :w],
                        w2row[:, c * H + bh * 512: c * H + bh * 512 + 512],
                        start=True, stop=True, skip_group_check=True,
                        tile_position=(0, pos),
                    )
                if si < 3:
                    nc.vector.tensor_copy(
                        w2slot[si][:, bh * 512:(bh + 1) * 512], pt[:])
                else:
                    nc.scalar.copy(w2slot[si][:, bh * 512:(bh + 1) * 512],
                                   pt[:])

        build_w2slot(0)
        build_w2slot(1)
        build_w2slot(2)

        # --- main loop over slots ---
        # Consecutive matmul groups alternate psum banks (A,B,A,B...): a
        # group accumulating onto the same bank as its predecessor stalls
        # on the psum RAW chain (~430ns/group); alternating banks lets
        # fill(i+1) overlap drain(i) (~2x faster group cadence).
        for si, slot in enumerate(slots):
            ps = [pp.tile([128, 512], f32, name=f"ps{si}_0", tag="psA"),
                  pp.tile([128, 512], f32, name=f"ps{si}_1", tag="psB")]
            # bias: psum[t, h] = S*b1[c][bh*512 + h] via K=1 ones matmul
            for bh in range(2):
                for (c, lo, w, pos) in slot:
                    nc.tensor.matmul(
                        ps[bh][pos:pos + w, :],
                        ones[:, :w],
                        b1t[:, c * H + bh * 512: c * H + bh * 512 + 512],
                        start=True, stop=False, skip_group_check=True,
                        tile_position=(0, pos),
                    )
            # layer 1: z.T @ W1 k-chunks; chunk matmuls in a group target
            # disjoint psum partition ranges -> col-tiled concurrency
            for k in range(KC):
                for bh in range(2):
                    for (c, lo, w, pos) in slot:
                        w1t = w1ts[c // 2]
                        nc.tensor.matmul(
                            ps[bh][pos:pos + w, :],
                            zt[:, k, c * TW + lo: c * TW + lo + w],
                            w1t[:, c % 2, k, bh * 512:bh * 512 + 512],
                            start=False, stop=(k == KC - 1),
                            skip_group_check=True, tile_position=(0, pos),
                        )
            for bh in range(2):
                # gelu (undo the fp8 scale), then layer 2 against the
                # franken W2 tile: out[t] = sum_h g[t,h] * W2[cid(t),h].
                # (tensor_tensor_reduce is a custom DVE op that crashes
                # this runtime; use the native mult + reduce pair.)
                g = gp.tile([128, 512], f16, name=f"g{si}_{bh}", tag="g")
                nc.scalar.activation(g[:], ps[bh][:], gelu, scale=1.0 / S)
                sc = scr.tile([128, 512], f16, name=f"sc{si}_{bh}", tag="sc")
                nc.vector.tensor_tensor(
                    out=sc[:], in0=g[:],
                    in1=w2slot[si][:, bh * 512:(bh + 1) * 512], op=mult)
                # partial reduce 512 -> 16 in fp16 (2x DVE mode; DVE sums
                # each 32-group in fp32 internally); host finishes in fp32
                col = 2 * si + bh
                with nc.allow_low_precision("host sums the 16 partials"):
                    nc.vector.tensor_reduce(
                        out=stage2[:, col * 16:(col + 1) * 16],
                        in_=sc[:].rearrange("p (a b) -> p a b", b=32),
                        axis=mybir.AxisListType.X, op=add)
            # next slots' W2 tiles ride inside the matmul stream
            if si + 3 < len(slots):
                build_w2slot(si + 3)
            # drain finished columns early so only the last slot's output
            # store sits in the tail
            if si == len(slots) - 2:
                nc.gpsimd.dma_start(out=out_d[:, :(2 * si + 2) * 16],
                                    in_=stage2[:, :(2 * si + 2) * 16])

        nc.gpsimd.dma_start(out=out_d[:, (2 * len(slots) - 2) * 16:],
                            in_=stage2[:, (2 * len(slots) - 2) * 16:])

    nc.finalize()
    return nc


def _get_compiled(TW):
    if TW not in _COMPILED:
        _COMPILED[TW] = _build(TW)
    return _COMPILED[TW]


def kernel(z, company_id, W1, b1, W2, b2):
    import ml_dtypes
    from concourse.bass_utils import run_bass_kernel_spmd

    e3 = ml_dtypes.float8_e3m4

    z = np.asarray(z, dtype=np.float32)
    cid = np.asarray(company_id).astype(np.int64).ravel()
    W1 = np.asarray(W1, dtype=np.float32)
    b1 = np.asarray(b1, dtype=np.float32)
    W2 = np.asarray(W2, dtype=np.float32)
    b2 = np.asarray(b2, dtype=np.float32)
    O = W2.shape[2]

    idx_by_company = [np.nonzero(cid == gc)[0] for gc in range(C)]
    max_cnt = max(max((len(ix) for ix in idx_by_company), default=1), 1)
    TW = ((max_cnt + 31) // 32) * 32
    assert TW <= 128, f"company with {max_cnt} tokens unsupported"

    nc = _get_compiled(TW)
    entries = _entries(TW)
    NPAIR = (CPC + 1) // 2

    in_maps = []
    for core in range(NCORES):
        zt = np.zeros((128, KC, CPC * TW), dtype=np.float16)
        for ci in range(CPC):
            gc = core * CPC + ci
            ix = idx_by_company[gc]
            if len(ix) == 0:
                continue
            # [cnt, D] -> [128, KC, cnt]
            zc = z[ix].astype(np.float16).reshape(len(ix), KC, 128)
            zt[:, :, ci * TW: ci * TW + len(ix)] = zc.transpose(2, 1, 0)
        w1 = np.zeros((NPAIR, 128, 2 * KC * H), dtype=e3)
        for ci in range(CPC):
            gc = core * CPC + ci
            # [D, H] -> [KC, 128, H] -> [128, KC*H]
            w1c = np.clip(W1[gc] * S, -15.5, 15.5).reshape(KC, 128, H)
            w1[ci // 2, :, (ci % 2) * KC * H: (ci % 2 + 1) * KC * H] = (
                w1c.transpose(1, 0, 2).reshape(128, KC * H).astype(e3)
            )
        cst = np.zeros((1, 2 * CPC * H), dtype=np.float16)
        cst[0, :CPC * H] = (b1[core * CPC:(core + 1) * CPC] * S).astype(
            np.float16).ravel()
        cst[0, CPC * H:] = W2[core * CPC:(core + 1) * CPC, :, 0].astype(
            np.float16).ravel()
        in_maps.append({
            "zt": np.ascontiguousarray(zt),
            "w1": np.ascontiguousarray(w1),
            "cst": np.ascontiguousarray(cst),
        })

    res = run_bass_kernel_spmd(nc, in_maps, list(range(NCORES)))

    out = np.zeros((B, O), dtype=np.float32)
    for core in range(NCORES):
        st = res.results[core]["out"].astype(np.float32)
        st = st.reshape(128, -1, 16).sum(2)  # [128, 2*NSLOT] fp32
        for (c, lo, w, pos, col) in entries:
            gc = core * CPC + c
            ix = idx_by_company[gc]
            n = min(w, len(ix) - lo)
            if n <= 0:
                continue
            out[ix[lo:lo + n], 0] = (st[pos:pos + n, 2 * col]
                                     + st[pos:pos + n, 2 * col + 1]
                                     + b2[gc, 0])
    return out


# revision 59
# speedup vs baseline: 1.0930x; 1.0930x over previous
"""Trainium2 Bass kernel for CompanySpecificHeads (MoE-style routed MLP heads).

Semantics (matching the reference):
    out[b] = gelu(z[b] @ W1[cid[b]] + b1[cid[b]]) @ W2[cid[b]] + b2[cid[b]]

Strategy: expert-parallel across 8 NeuronCores, 8 companies per core, with a
tokens-stationary PE mapping and fp8 weights:

  - W1 is streamed from HBM in fp8 e3m4 (scaled by 64, clipped to +-15.5),
    halving the dominant DMA traffic vs fp16. Measured end-to-end rel err
    ~1.3e-2 (gate 2e-2); z stays fp16 (mixed-dtype matmul is allowed).
  - Layer 1 runs tokens-stationary: the routed z tile [128d x <=64 tokens]
    is the stationary operand, W1[c] k-chunks stream as the N=512 moving
    operand into psum[tokens, h]. ~10 matmuls/company instead of 42, and no
    per-matmul 128-col weight LDWEIGHTS churn.
  - Token chunks of different companies sit at disjoint psum partition
    ranges (column tiling: 2 x 64-wide at pos 0/64, or 4 x 32-wide at pos
    0/32/64/96) so their W1 streams run concurrently on separate XBUSes.
  - b1 enters via a K=1 ones-matmul into psum before the k-chunks.
  - gelu: ACT reads psum with scale=1/64 (undoes the fp8 scaling).
  - Layer 2 (O=1): DVE tensor_tensor_reduce (g * W2rep, reduce-add along h)
    into a staging tile. DVE requires partition base 0/64, so each ttr
    covers a 64-row side with one company's W2 block; rows belonging to
    other companies are garbage and ignored by the host. W2 is replicated
    across partitions by GPSIMD partition_broadcast (idle engine).
  - Host scatters staged outputs back to [B, 1] and adds b2 in fp32.
"""

import numpy as np

B, C, D, H = 4096, 64, 512, 1024
NCORES = 8
CPC = C // NCORES  # companies per core
KC = D // 128      # contraction chunks of 128
S = 64.0           # fp8 weight scale

_COMPILED = {}


def _plan(TW):
    """Slot list; each slot is a list of entries (c, lo, w, pos).

    First-64 token chunks pair two companies at pos 0/64; overflow chunks
    (w<=32) pack four companies at pos 0/32/64/96, (32<w<=64) pack two.
    """
    slots = []
    wA = min(TW, 64)
    for c0 in range(0, CPC, 2):
        slots.append([(c0 + i, 0, wA, 64 * i) for i in range(2) if c0 + i < CPC])
    if TW > 64:
        wB = TW - 64
        if wB <= 32:
            for c0 in range(0, CPC, 4):
                slots.append([(c0 + i, 64, wB, 32 * i)
                              for i in range(4) if c0 + i < CPC])
        else:
            for c0 in range(0, CPC, 2):
                slots.append([(c0 + i, 64, wB, 64 * i)
                              for i in range(2) if c0 + i < CPC])
    return slots


def _entries(TW):
    """Flat (c, lo, w, pos, col) list; col = stage column = slot index."""
    out = []
    for si, slot in enumerate(_plan(TW)):
        for (c, lo, w, pos) in slot:
            out.append((c, lo, w, pos, si))
    return out


def _build(TW):
    import concourse.bass as bass
    import concourse.bacc as bacc
    import concourse.mybir as mybir
    from concourse.tile import TileContext
    from contextlib import ExitStack

    f32 = mybir.dt.float32
    f16 = mybir.dt.float16
    f8 = mybir.dt.float8e3

    slots = _plan(TW)
    NCOL = len(slots)
    NPAIR = (CPC + 1) // 2

    gelu = mybir.ActivationFunctionType.Gelu
    ident = mybir.ActivationFunctionType.Identity
    mult = mybir.AluOpType.mult
    add = mybir.AluOpType.add

    nc = bacc.Bacc(None, target_bir_lowering=False)

    # z tokens routed+padded per company: zt[p, k, c*TW + t] = z[tok, 128k+p]
    zt_d = nc.dram_tensor("zt", [128, KC, CPC * TW], f16, kind="ExternalInput")
    # W1 pair-major: w1[pr, p, cc*KC*H + k*H + h] = W1[2pr+cc, 128k+p, h]*S (e3m4)
    w1_d = nc.dram_tensor("w1", [NPAIR, 128, 2 * KC * H], f8, kind="ExternalInput")
    # consts on partition 0: [b1*S (c-major, CPC*H) | w2 (c-major, CPC*H)]
    cst_d = nc.dram_tensor("cst", [1, 2 * CPC * H], f16, kind="ExternalInput")
    out_d = nc.dram_tensor("out", [128, 2 * NCOL * 16], f16,
                           kind="ExternalOutput")

    with TileContext(nc) as tc, ExitStack() as ctx:
        const = ctx.enter_context(tc.tile_pool(name="const", bufs=1))

        # --- input DMAs ---
        # Everything on ONE HWDGE ring (sync), in need-order: two rings
        # do NOT share SDMA bandwidth fairly (the W1 ring starved z/cst
        # until ~20us), while a single ring is FIFO and hits full rate.
        cst = const.tile([1, 2 * CPC * H], f16)
        nc.sync.dma_start(out=cst[:], in_=cst_d[:])
        b1t = cst[:, 0:CPC * H]
        w2row = cst[:, CPC * H:]

        # z k0-1 now; k2-3 after the first W1 pair (slot 0 starts sooner)
        zt = const.tile([128, KC, CPC * TW], f16)
        nc.sync.dma_start(out=zt[:, :2], in_=zt_d[:, :2])

        # W1: one DMA per company pair (~1MB each, e3m4).
        w1ts = []
        for pr in range(NPAIR):
            w1t = const.tile([128, 2, KC, H], f8, name=f"w1_{pr}")
            nc.sync.dma_start(out=w1t[:], in_=w1_d[pr])
            w1ts.append(w1t)
            if pr == 0:
                nc.sync.dma_start(out=zt[:, 2:], in_=zt_d[:, 2:])

        # --- small constants on-chip ---
        ones = const.tile([1, 128], f16)
        nc.gpsimd.memset(ones[:], 1.0)
        wsc = const.tile([128, 128], f16)
        nc.gpsimd.memset(wsc[:], 0.0)
        wsc2 = const.tile([128, 512], f16)
        nc.gpsimd.memset(wsc2[:], 0.0)

        # 16 fp16 partial sums per (slot, h-half); host finishes the sum
        stage2 = const.tile([128, 2 * NCOL * 16], f16)

        # pools — PSUM: psA(3) + psB(3) + pt(2) tiles of one bank each = 8 banks
        pp = ctx.enter_context(tc.tile_pool(name="pp", bufs=3, space="PSUM"))
        gp = ctx.enter_context(tc.tile_pool(name="gp", bufs=8))
        scr = ctx.enter_context(tc.tile_pool(name="scr", bufs=6))

        # --- PE warmup: keep the PE busy until the first W1 pair lands so
        # the HAM clock-gate reaches (and holds) the warm 2.4GHz state.
        # Dependency-free N=128 matmuls on memset scratch (~214ns each cold
        # including the per-matmul LDWEIGHTS).
        # N=512 keeps the PE ~80% busy despite the per-matmul LDWEIGHTS, so
        # the HAM activity window actually trips before the main stream.
        wp = pp.tile([128, 512], f32, tag="pt", bufs=2)
        for _ in range(10):
            nc.tensor.matmul(wp[:], wsc[:], wsc2[:], start=True, stop=True)

        # --- per-slot W2 tiles: w2slot[s][pos:pos+w, h] = W2[c, h] ---
        # Replicate each company's W2 row across its slot's psum partition
        # range with a K=1 ones-matmul at that tile position (the only
        # partition-range replication that works at runtime), then one
        # full-width DVE copy psum -> SBUF. Construction for slot s+1 is
        # emitted inside the main stream (after slot s's matmuls) so the
        # psum->SBUF copy dependencies never stall the PE queue head.
        w2slot = [const.tile([128, H], f16, name=f"w2slot_{si}")
                  for si in range(len(slots))]

        def build_w2slot(si):
            # early slots' psum->SBUF copies go to DVE (idle before the
            # stream); later slots' to ACT (DVE is the mid-stream critical
            # path, ACT has slack)
            slot = slots[si]
            for bh in range(2):
                pt = pp.tile([128, 512], f32, name=f"w2p{si}_{bh}", tag="pt",
                             bufs=2)
                for (c, lo, w, pos) in slot:
                    nc.tensor.matmul(
                        pt[pos:pos + w, :],
                        ones[:, :w],
                        w2row[:, c * H + bh * 512: c * H + bh * 512 + 512],
                        start=True, stop=True, skip_group_check=True,
                        tile_position=(0, pos),
                    )
                if si < 3:
                    nc.vector.tensor_copy(
                        w2slot[si][:, bh * 512:(bh + 1) * 512], pt[:])
                else:
                    nc.scalar.copy(w2slot[si][:, bh * 512:(bh + 1) * 512],
                                   pt[:])

        build_w2slot(0)
        build_w2slot(1)
        build_w2slot(2)

        # --- main loop over slots ---
        # Per h-half (bank): bias matmul + 4 k-chunk matmuls accumulate,
        # then gelu (ACT) -> W2-mult -> reduce (DVE) drain that bank while
        # the other bank's matmuls stream. (Interleaving the two banks'
        # k-groups was measured strictly slower, as was putting the mult
        # on GPSIMD — keep this per-bank-serial, DVE-drain form.)
        for si, slot in enumerate(slots):
            for bh in range(2):
                ps = pp.tile([128, 512], f32, name=f"ps{si}_{bh}",
                             tag=("psA", "psB")[bh])
                # bias: psum[t, h] = S*b1[c][bh*512 + h] via K=1 ones matmul
                for (c, lo, w, pos) in slot:
                    nc.tensor.matmul(
                        ps[pos:pos + w, :],
                        ones[:, :w],
                        b1t[:, c * H + bh * 512: c * H + bh * 512 + 512],
                        start=True, stop=False, skip_group_check=True,
                        tile_position=(0, pos),
                    )
                # layer 1: z.T @ W1 k-chunks; chunk matmuls in a slot target
                # disjoint psum partition ranges -> col-tiled concurrency
                for k in range(KC):
                    for (c, lo, w, pos) in slot:
                        w1t = w1ts[c // 2]
                        nc.tensor.matmul(
                            ps[pos:pos + w, :],
                            zt[:, k, c * TW + lo: c * TW + lo + w],
                            w1t[:, c % 2, k, bh * 512:bh * 512 + 512],
                            start=False, stop=(k == KC - 1),
                            skip_group_check=True, tile_position=(0, pos),
                        )
                # gelu (undo the fp8 scale), then layer 2 against the
                # franken W2 tile: out[t] = sum_h g[t,h] * W2[cid(t),h].
                # (tensor_tensor_reduce is a custom DVE op that crashes
                # this runtime; use the native mult + reduce pair.)
                g = gp.tile([128, 512], f16, name=f"g{si}_{bh}", tag="g")
                nc.scalar.activation(g[:], ps[:], gelu, scale=1.0 / S)
                sc = scr.tile([128, 512], f16, name=f"sc{si}_{bh}", tag="sc")
                nc.vector.tensor_tensor(
                    out=sc[:], in0=g[:],
                    in1=w2slot[si][:, bh * 512:(bh + 1) * 512], op=mult)
                # partial reduce 512 -> 16 in fp16 (2x DVE mode; DVE sums
                # each 32-group in fp32 internally); host finishes in fp32
                col = 2 * si + bh
                with nc.allow_low_precision("host sums the 16 partials"):
                    nc.vector.tensor_reduce(
                        out=stage2[:, col * 16:(col + 1) * 16],
                        in_=sc[:].rearrange("p (a b) -> p a b", b=32),
                        axis=mybir.AxisListType.X, op=add)
            # next slots' W2 tiles ride inside the matmul stream
            if si + 3 < len(slots):
                build_w2slot(si + 3)
            # drain finished columns early so only the last slot's output
            # store sits in the tail
            if si == len(slots) - 2:
                nc.gpsimd.dma_start(out=out_d[:, :(2 * si + 2) * 16],
                                    in_=stage2[:, :(2 * si + 2) * 16])

        nc.gpsimd.dma_start(out=out_d[:, (2 * len(slots) - 2) * 16:],
                            in_=stage2[:, (2 * len(slots) - 2) * 16:])

    nc.finalize()
    return nc


def _get_compiled(TW):
    if TW not in _COMPILED:
        _COMPILED[TW] = _build(TW)
    return _COMPILED[TW]


def kernel(z, company_id, W1, b1, W2, b2):
    import ml_dtypes
    from concourse.bass_utils import run_bass_kernel_spmd

    e3 = ml_dtypes.float8_e3m4

    z = np.asarray(z, dtype=np.float32)
    cid = np.asarray(company_id).astype(np.int64).ravel()
    W1 = np.asarray(W1, dtype=np.float32)
    b1 = np.asarray(b1, dtype=np.float32)
    W2 = np.asarray(W2, dtype=np.float32)
    b2 = np.asarray(b2, dtype=np.float32)
    O = W2.shape[2]

    idx_by_company = [np.nonzero(cid == gc)[0] for gc in range(C)]
    max_cnt = max(max((len(ix) for ix in idx_by_company), default=1), 1)
    TW = ((max_cnt + 31) // 32) * 32
    assert TW <= 128, f"company with {max_cnt} tokens unsupported"

    nc = _get_compiled(TW)
    entries = _entries(TW)
    NPAIR = (CPC + 1) // 2

    in_maps = []
    for core in range(NCORES):
        zt = np.zeros((128, KC, CPC * TW), dtype=np.float16)
        for ci in range(CPC):
            gc = core * CPC + ci
            ix = idx_by_company[gc]
            if len(ix) == 0:
                continue
            # [cnt, D] -> [128, KC, cnt]
            zc = z[ix].astype(np.float16).reshape(len(ix), KC, 128)
            zt[:, :, ci * TW: ci * TW + len(ix)] = zc.transpose(2, 1, 0)
        w1 = np.zeros((NPAIR, 128, 2 * KC * H), dtype=e3)
        for ci in range(CPC):
            gc = core * CPC + ci
            # [D, H] -> [KC, 128, H] -> [128, KC*H]
            w1c = np.clip(W1[gc] * S, -15.5, 15.5).reshape(KC, 128, H)
            w1[ci // 2, :, (ci % 2) * KC * H: (ci % 2 + 1) * KC * H] = (
                w1c.transpose(1, 0, 2).reshape(128, KC * H).astype(e3)
            )
        cst = np.zeros((1, 2 * CPC * H), dtype=np.float16)
        cst[0, :CPC * H] = (b1[core * CPC:(core + 1) * CPC] * S).astype(
            np.float16).ravel()
        cst[0, CPC * H:] = W2[core * CPC:(core + 1) * CPC, :, 0].astype(
            np.float16).ravel()
        in_maps.append({
            "zt": np.ascontiguousarray(zt),
            "w1": np.ascontiguousarray(w1),
            "cst": np.ascontiguousarray(cst),
        })

    res = run_bass_kernel_spmd(nc, in_maps, list(range(NCORES)))

    out = np.zeros((B, O), dtype=np.float32)
    for core in range(NCORES):
        st = res.results[core]["out"].astype(np.float32)
        st = st.reshape(128, -1, 16).sum(2)  # [128, 2*NSLOT] fp32
        for (c, lo, w, pos, col) in entries:
            gc = core * CPC + c
            ix = idx_by_company[gc]
            n = min(w, len(ix) - lo)
            if n <= 0:
                continue
            out[ix[lo:lo + n], 0] = (st[pos:pos + n, 2 * col]
                                     + st[pos:pos + n, 2 * col + 1]
                                     + b2[gc, 0])
    return out
